# revision 33
# baseline (speedup 1.0000x reference)
"""Trainium2 Bass kernel for nn_LogisticRegressionModel (polynomial-feature logistic regression).

Math: reference computes sigmoid(poly_features(x) @ W.T + b), poly features = all
monomials of x (dim 16) up to degree 4, soft-weighted per degree. Every monomial
embeds as a degree-4 monomial over x1 = [x, 1] (17 symbols). Folding W, b, M_raw
into a symmetric quartic matrix over the 153 wrap-encoded unordered pairs
p=(d,j) <-> {j,(j+d)%17}: logit_i = XX_i^T S153 XX_i with XX_i[p] = x1_i[a] x1_i[b].

Device pipeline (feature-major layout, per 512-sample group, fp16 / fp32 PSUM):
  XX^T[p, s] = X9[p, s] * X9s[p, s]        -- DVE, X9/X9s host-replicated x1^T rows
  Z = U^T XX^T  (S153 = U diag(sign) U^T)  -- matmuls, stationary U resident
  P = Z^2                                  -- ScalarE Square, PSUM -> SBUF
  q = sign^T P                             -- matmuls (K=128 + K=25 bands)
  out = sigmoid(q)                         -- ScalarE, batched per 3-group window
Perf notes (tuned against the TRN2 timeline cost model):
  * 153 = 128 + 25. The 25-row tail chunks are zero-PADDED to K=128 (host pads
    U with zero rows; xx1 tile tails zeroed once by GPSIMD) so consecutive
    matmuls keep one PE tile_size -- a tile-size switch costs ~95ns of PE
    pipeline restart and blocks the 1.2->2.4 GHz ramp.
  * Matmuls are emitted in same-tile-size blocks; the 25-row z1/q outputs pack
    into 32-partition PSUM bands at bases 0/32/64 (3 groups per window), and
    band-tiled (32,32)/(128,32) matmuls pack in the PE array (~4ns apart).
  * Each window's q-matmuls/sigmoid are deferred into the NEXT window so
    ScalarE squares never stall the PE; outputs stream per window.
  * 10 warm-up matmuls on the constant tile run during the input DMA wait to
    pre-ramp the PE clock. DMA instruction count is minimized (packed pair
    tensors, 4+1 slices, packed constants) -- each DMA costs ~625ns on the
    serialized HWDGE pipe plus bytes at 360 GB/s on the shared DMA device.

Sharding: pure data-parallel over the batch, 4096 rows per core x 8 cores.
"""
import sys
import numpy as np
from itertools import combinations_with_replacement, permutations

sys.path.insert(0, "/opt/trn_rl_repo")

import concourse.bass as bass
import concourse.bacc as bacc
import concourse.tile as tile
from concourse import mybir
from concourse import bass_utils

BATCH = 32768
D = 16
DA = 17            # features + constant slot
ND = 9             # wrap distances 0..8
PD = ND * DA       # 153 unordered pairs
K0, K1 = 128, PD - 128
MAX_DEGREE = 4
N_CORES = 8
B_CORE = BATCH // N_CORES   # 4096
GW = 512                    # group width (PSUM bank = 512 fp32)
N_GROUPS = B_CORE // GW     # 8
WINDOWS = [[0, 1, 2], [3, 4, 5], [6, 7]]
NCOL = 310                  # packed const columns: 153 u0 | 153 u1 | sg0 | sg1
P_FULL = 1 + sum(
    len(list(combinations_with_replacement(range(D), d))) for d in range(1, MAX_DEGREE + 1)
)

# wrap pair tables (row p of XX^T multiplies x1 rows PAIR_A[p] * PAIR_B[p])
PAIR_A = np.array([j for d in range(ND) for j in range(DA)], np.int64)
PAIR_B = np.array([(j + d) % DA for d in range(ND) for j in range(DA)], np.int64)


def _build_s153(W, b, M_raw):
    """Fold W, b and the soft degree weights into the symmetric quartic
    coefficient matrix over the 153 wrap-encoded unordered pairs."""
    W = np.asarray(W, np.float64)
    bval = float(np.asarray(b).reshape(-1)[0])
    M = 1.0 / (1.0 + np.exp(-float(np.asarray(M_raw)))) * (MAX_DEGREE - 1) + 1.0
    coef = {(16, 16, 16, 16): float(W[0, 0]) + bval}
    col = 1
    for d in range(1, MAX_DEGREE + 1):
        w_d = 1.0 / (1.0 + np.exp(-10.0 * (M - d + 0.5)))
        for t in combinations_with_replacement(range(D), d):
            tup = tuple(sorted(t + (16,) * (4 - d)))
            coef[tup] = float(W[0, col]) * w_d
            col += 1
    assert col == P_FULL
    S4 = np.zeros((DA * DA, DA * DA), np.float64)
    for tup, c in coef.items():
        perms = set(permutations(tup))
        v = c / len(perms)
        for (a, b2, c2, d2) in perms:
            S4[a * DA + b2, c2 * DA + d2] += v
    lookup = {}
    for p, (a, c) in enumerate(zip(PAIR_A, PAIR_B)):
        lookup[(a, c)] = p
        lookup[(c, a)] = p
    B = np.zeros((DA * DA, PD))
    for j in range(DA):
        for k in range(DA):
            B[j * DA + k, lookup[(j, k)]] = 1.0
    return B.T @ S4 @ B  # float64 [153, 153]


def _build_const(S):
    """Eigendecompose S153 and pack U + sign vectors into one [128, 310] fp16."""
    lam, V = np.linalg.eigh(S)
    U = (V * np.sqrt(np.abs(lam))[None, :]).astype(np.float16)  # columns scaled
    sign = np.sign(lam).astype(np.float16)
    cst = np.zeros((K0, NCOL), np.float16)
    cst[:, :PD] = U[:K0]                    # u0 [128, 153]
    cst[:K1, PD:2 * PD] = U[K0:]            # u1 [25, 153]
    cst[:, 306] = sign[:K0]                 # sg0
    for gp in range(3):                     # sg1 banded at 32*gp
        cst[32 * gp : 32 * gp + K1, 307] = sign[K0:]
    return cst


def _build_nc():
    nc = bacc.Bacc("TRN2", target_bir_lowering=False, debug=False, enable_asserts=False)
    f16 = mybir.dt.float16
    f32 = mybir.dt.float32
    # packed pair operands: [:, 0, :] = X9 rows, [:, 1, :] = X9s rows
    pa_d = nc.dram_tensor("pa", [K0, 2, B_CORE], f16, kind="ExternalInput").ap()
    pb_d = nc.dram_tensor("pb", [K1, 2, B_CORE], f16, kind="ExternalInput").ap()
    cst_d = nc.dram_tensor("cst", [K0, NCOL], f16, kind="ExternalInput").ap()
    out_d = nc.dram_tensor("out", [N_GROUPS, GW], f32, kind="ExternalOutput").ap()

    with tile.TileContext(nc) as tc:
        with (
            tc.tile_pool(name="const", bufs=1) as const_pool,
            tc.tile_pool(name="xx", bufs=6) as xx_pool,
            tc.tile_pool(name="p0", bufs=8) as p0_pool,
            tc.tile_pool(name="z0ps", bufs=4, space="PSUM") as z0_pool,
            tc.tile_pool(name="z1ps", bufs=1, space="PSUM") as z1_pool,
            tc.tile_pool(name="qps", bufs=1, space="PSUM") as q_pool,
        ):
            # resident constants + staged inputs
            cst = const_pool.tile([K0, NCOL], f16)
            u0 = cst[:, 0:PD]
            u1 = cst[:K1, PD:2 * PD]
            u1p = cst[:, PD:2 * PD]   # rows 25..127 are zeros (host-padded)
            sg0 = cst[:, 306:307]
            sg1 = cst[:, 307:308]

            pa = const_pool.tile([K0, 2, B_CORE], f16)
            pb = const_pool.tile([K1, 2, B_CORE], f16)
            # progressive slices sized so slice k lands before DVE consumes it;
            # cst rides after the first pair (needed by the first matmul)
            nc.sync.dma_start(out=cst[:], in_=cst_d[:])
            # window-aligned slices: each window's data in one DMA pair, so no
            # mid-window stalls and only 7 DMA instructions total
            for lo, hi in ((0, 3 * GW), (3 * GW, 6 * GW), (6 * GW, 8 * GW)):
                nc.sync.dma_start(out=pa[:, :, lo:hi], in_=pa_d[:, :, lo:hi])
                nc.sync.dma_start(out=pb[:, :, lo:hi], in_=pb_d[:, :, lo:hi])

            # pre-zero the xx1 rotation buffers' tail rows once: the padded
            # K=128 matmuls read rows 25..127 against zero weights
            for i in range(6):
                xx1z = xx_pool.tile([K0, GW], f16, name="xx1")
                nc.gpsimd.memset(xx1z[:], 0.0)

            # warm the sigmoid table-set early (Square co-resides in every set)
            warm = const_pool.tile([1, 1], f32)
            nc.vector.memset(warm[:], 0.0)
            nc.scalar.activation(warm[:], warm[:], mybir.ActivationFunctionType.Sigmoid)

            # banded PSUM tiles (bands at 0/32/64); zero once so gaps are defined
            z1_tiles = [z1_pool.tile([64 + K1, GW], f32, name=f"z1t{i}") for i in range(2)]
            q_tiles = [q_pool.tile([65, GW], f32, name=f"qt{i}") for i in range(2)]
            for t in z1_tiles + q_tiles:
                nc.vector.memset(t[:], 0.0)

            p1_tiles = [const_pool.tile([64 + K1, GW], f16, name=f"p1t{i}") for i in range(2)]
            o_all = const_pool.tile([65, len(WINDOWS) * GW], f32)

            # warm-up matmuls bridge the input-DMA wait so the PE clock is
            # ramped (and stays ramped) when the first real matmul issues
            warm_ps = z0_pool.tile([K0, 256], f32, name="z0_ps")
            for _ in range(18):
                nc.tensor.matmul(out=warm_ps[:], lhsT=cst[:, :K0],
                                 rhs=cst[:, :256], start=True, stop=True,
                                 skip_group_check=True)
            for _ in range(8):
                nc.tensor.matmul(out=warm_ps[:, :128], lhsT=cst[:, :K0],
                                 rhs=cst[:, :128], start=True, stop=True,
                                 skip_group_check=True)

            p0_win = {}

            def window_mms(wi, prev_wi):
                """Window wi's matmuls in same-tile-size blocks (each tile-size
                switch costs ~95ns of PE pipeline restart), with the previous
                window's q-matmuls folded into the matching blocks."""
                win = WINDOWS[wi]
                z1_ps = z1_tiles[wi % 2]
                z0s, xxs = [], []
                for gp, g in enumerate(win):
                    sl = slice(g * GW, (g + 1) * GW)
                    xx0 = xx_pool.tile([K0, GW], f16, name="xx0")
                    xx1 = xx_pool.tile([K0, GW], f16, name="xx1")
                    nc.vector.tensor_tensor(
                        out=xx0[:], in0=pa[:, 0, sl], in1=pa[:, 1, sl],
                        op=mybir.AluOpType.mult)
                    nc.vector.tensor_tensor(
                        out=xx1[:K1, :], in0=pb[:, 0, sl], in1=pb[:, 1, sl],
                        op=mybir.AluOpType.mult)
                    xxs.append((xx0, xx1))
                    z0s.append(z0_pool.tile([K0, GW], f32, name="z0_ps"))
                if prev_wi is not None:
                    pwin = WINDOWS[prev_wi]
                    pq_ps = q_tiles[prev_wi % 2]
                    pp1 = p1_tiles[prev_wi % 2]
                # block A: prev window's q K1-parts, tile (32,32) -- opens q
                if prev_wi is not None:
                    for gp in range(len(pwin)):
                        band = slice(32 * gp, 32 * gp + K1)
                        nc.tensor.matmul(out=pq_ps[32 * gp : 32 * gp + 1, :],
                                         lhsT=sg1[band, :], rhs=pp1[band, :],
                                         start=True, stop=False,
                                         skip_group_check=True)
                def blk_z0():
                    for gp in range(len(win)):
                        nc.tensor.matmul(out=z0s[gp][:], lhsT=u0[:, :K0],
                                         rhs=xxs[gp][0][:], start=True, stop=False,
                                         skip_group_check=True)
                    for gp in range(len(win)):
                        nc.tensor.matmul(out=z0s[gp][:], lhsT=u1p[:, :K0],
                                         rhs=xxs[gp][1][:], start=False, stop=True,
                                         skip_group_check=True)

                def blk_z1():
                    for gp in range(len(win)):
                        band = slice(32 * gp, 32 * gp + K1)
                        nc.tensor.matmul(out=z1_ps[band, :], lhsT=u1p[:, K0:],
                                         rhs=xxs[gp][1][:], start=True, stop=False,
                                         skip_group_check=True)
                    for gp in range(len(win)):
                        band = slice(32 * gp, 32 * gp + K1)
                        nc.tensor.matmul(out=z1_ps[band, :], lhsT=u0[:, K0:],
                                         rhs=xxs[gp][0][:], start=False, stop=True,
                                         skip_group_check=True)

                def blk_pq0():
                    if prev_wi is not None:
                        for gp in range(len(pwin)):
                            nc.tensor.matmul(out=pq_ps[32 * gp : 32 * gp + 1, :],
                                             lhsT=sg0, rhs=p0_win.pop(pwin[gp])[:],
                                             start=False, stop=True,
                                             skip_group_check=True)

                if wi == len(WINDOWS) - 1:
                    # last window: z1 first so its ScalarE square (gating the
                    # drain-phase q-matmuls) starts as early as possible
                    blk_z1(); blk_z0(); blk_pq0()
                else:
                    blk_z0(); blk_z1(); blk_pq0()
                # ScalarE: square z1 first in late windows (drain-phase q needs
                # it), then this window's z0 chunks
                late = wi == len(WINDOWS) - 1
                if late:
                    nc.scalar.activation(p1_tiles[wi % 2][:], z1_ps[:],
                                         mybir.ActivationFunctionType.Square)
                for gp, g in enumerate(win):
                    p0_sb = p0_pool.tile([K0, GW], f16, name="p0")
                    nc.scalar.activation(p0_sb[:], z0s[gp][:],
                                         mybir.ActivationFunctionType.Square)
                    p0_win[g] = p0_sb
                if not late:
                    nc.scalar.activation(p1_tiles[wi % 2][:], z1_ps[:],
                                         mybir.ActivationFunctionType.Square)
                # prev window's sigmoid after its q bands close, then store
                if prev_wi is not None:
                    nc.scalar.activation(
                        o_all[:, prev_wi * GW:(prev_wi + 1) * GW], pq_ps[:],
                        mybir.ActivationFunctionType.Sigmoid)
                    for gp2 in range(len(pwin)):
                        nc.sync.dma_start(
                            out=out_d[pwin[gp2] : pwin[gp2] + 1, :],
                            in_=o_all[32 * gp2 : 32 * gp2 + 1,
                                      prev_wi * GW:(prev_wi + 1) * GW])

            def finish_last(wi):
                """Final window's q-matmuls + sigmoid (no next window to ride)."""
                win = WINDOWS[wi]
                q_ps = q_tiles[wi % 2]
                p1_sb = p1_tiles[wi % 2]
                for gp in range(len(win)):
                    band = slice(32 * gp, 32 * gp + K1)
                    nc.tensor.matmul(out=q_ps[32 * gp : 32 * gp + 1, :],
                                     lhsT=sg1[band, :], rhs=p1_sb[band, :],
                                     start=True, stop=False, skip_group_check=True)
                for gp in range(len(win)):
                    nc.tensor.matmul(out=q_ps[32 * gp : 32 * gp + 1, :],
                                     lhsT=sg0, rhs=p0_win.pop(win[gp])[:],
                                     start=False, stop=True, skip_group_check=True)
                nc.scalar.activation(o_all[:, wi * GW:(wi + 1) * GW], q_ps[:],
                                     mybir.ActivationFunctionType.Sigmoid)
                for gp2 in range(len(win)):
                    nc.sync.dma_start(
                        out=out_d[win[gp2] : win[gp2] + 1, :],
                        in_=o_all[32 * gp2 : 32 * gp2 + 1,
                                  wi * GW:(wi + 1) * GW])

            for wi in range(len(WINDOWS)):
                window_mms(wi, wi - 1 if wi > 0 else None)
            finish_last(len(WINDOWS) - 1)

    nc.compile()
    return nc


_NC_CACHE = None


def _pack_x(x):
    """Per-core packed fp16 pair operands: pa [128, 2, B] rows 0..127 of
    (X9|X9s), pb [25, 2, B] rows 128..152."""
    x1 = np.concatenate([x, np.ones((x.shape[0], 1), np.float32)], axis=1)
    x1t = np.ascontiguousarray(x1.reshape(N_CORES, B_CORE, DA).transpose(0, 2, 1))
    x1t = x1t.astype(np.float16)  # [C, 17, B_CORE]
    X9 = x1t[:, PAIR_A, :]   # [C, 153, B]
    X9s = x1t[:, PAIR_B, :]
    pa = np.ascontiguousarray(np.stack([X9[:, :K0], X9s[:, :K0]], axis=2))
    pb = np.ascontiguousarray(np.stack([X9[:, K0:], X9s[:, K0:]], axis=2))
    return pa, pb  # [C,128,2,B], [C,25,2,B]


def _make_in_maps(x, W, b, M_raw):
    x = np.asarray(x, np.float32)
    pa, pb = _pack_x(x)
    cst = _build_const(_build_s153(W, b, M_raw))
    return [{"pa": pa[i], "pb": pb[i], "cst": cst} for i in range(N_CORES)]


def kernel(x, W, b, M_raw):
    global _NC_CACHE
    in_maps = _make_in_maps(x, W, b, M_raw)
    if _NC_CACHE is None:
        _NC_CACHE = _build_nc()
    nc = _NC_CACHE
    res = bass_utils.run_bass_kernel_spmd(nc, in_maps, core_ids=list(range(N_CORES)))
    out = np.concatenate([res.results[i]["out"].reshape(B_CORE) for i in range(N_CORES)])
    return out.reshape(BATCH, 1).astype(np.float32)


if __name__ == "__main__":
    x = np.random.randn(BATCH, D).astype(np.float32)
    W = (np.random.randn(1, P_FULL) * 0.02).astype(np.float32)
    b = np.zeros((1,), np.float32)
    M_raw = np.zeros((), np.float32)
    out = kernel(x, W, b, M_raw)
    print("out shape:", out.shape, out.dtype, out[:4, 0])


# revision 34
# speedup vs baseline: 1.0332x; 1.0332x over previous
"""Trainium2 Bass kernel for nn_LogisticRegressionModel (polynomial-feature logistic regression).

Math: reference computes sigmoid(poly_features(x) @ W.T + b), poly features = all
monomials of x (dim 16) up to degree 4, soft-weighted per degree. Every monomial
embeds as a degree-4 monomial over x1 = [x, 1] (17 symbols). Folding W, b, M_raw
into a symmetric quartic matrix over the 153 wrap-encoded unordered pairs
p=(d,j) <-> {j,(j+d)%17}: logit_i = XX_i^T S153 XX_i with XX_i[p] = x1_i[a] x1_i[b].

Device pipeline (feature-major layout, per 512-sample group, fp16 / fp32 PSUM):
  XX^T[p, s] = X9[p, s] * X9s[p, s]        -- DVE, X9/X9s host-replicated x1^T rows
  Z = U^T XX^T  (S153 = U diag(sign) U^T)  -- matmuls, stationary U resident
  P = Z^2                                  -- ScalarE Square, PSUM -> SBUF
  q = sign^T P                             -- matmuls (K=128 + K=25 bands)
  out = sigmoid(q)                         -- ScalarE, batched per 3-group window
Perf notes (tuned against the TRN2 timeline cost model):
  * 153 = 128 + 25. The 25-row tail chunks are zero-PADDED to K=128 (host pads
    U with zero rows; xx1 tile tails zeroed once by GPSIMD) so consecutive
    matmuls keep one PE tile_size -- a tile-size switch costs ~95ns of PE
    pipeline restart and blocks the 1.2->2.4 GHz ramp.
  * Matmuls are emitted in same-tile-size blocks; the 25-row z1/q outputs pack
    into 32-partition PSUM bands at bases 0/32/64 (3 groups per window), and
    band-tiled (32,32)/(128,32) matmuls pack in the PE array (~4ns apart).
  * Each window's q-matmuls/sigmoid are deferred into the NEXT window so
    ScalarE squares never stall the PE; outputs stream per window.
  * 10 warm-up matmuls on the constant tile run during the input DMA wait to
    pre-ramp the PE clock. DMA instruction count is minimized (packed pair
    tensors, 4+1 slices, packed constants) -- each DMA costs ~625ns on the
    serialized HWDGE pipe plus bytes at 360 GB/s on the shared DMA device.

Sharding: pure data-parallel over the batch, 4096 rows per core x 8 cores.
"""
import sys
import numpy as np
from itertools import combinations_with_replacement, permutations

sys.path.insert(0, "/opt/trn_rl_repo")

import concourse.bass as bass
import concourse.bacc as bacc
import concourse.tile as tile
from concourse import mybir
from concourse import bass_utils

BATCH = 32768
D = 16
DA = 17            # features + constant slot
ND = 9             # wrap distances 0..8
PD = ND * DA       # 153 unordered pairs
K0, K1 = 128, PD - 128
MAX_DEGREE = 4
N_CORES = 8
B_CORE = BATCH // N_CORES   # 4096
GW = 512                    # group width (PSUM bank = 512 fp32)
N_GROUPS = B_CORE // GW     # 8
WINDOWS = [[0, 1], [2, 3, 4], [5, 6, 7]]
NCOL = 310                  # packed const columns: 153 u0 | 153 u1 | sg0 | sg1
P_FULL = 1 + sum(
    len(list(combinations_with_replacement(range(D), d))) for d in range(1, MAX_DEGREE + 1)
)

# wrap pair tables (row p of XX^T multiplies x1 rows PAIR_A[p] * PAIR_B[p])
PAIR_A = np.array([j for d in range(ND) for j in range(DA)], np.int64)
PAIR_B = np.array([(j + d) % DA for d in range(ND) for j in range(DA)], np.int64)


def _build_s153(W, b, M_raw):
    """Fold W, b and the soft degree weights into the symmetric quartic
    coefficient matrix over the 153 wrap-encoded unordered pairs."""
    W = np.asarray(W, np.float64)
    bval = float(np.asarray(b).reshape(-1)[0])
    M = 1.0 / (1.0 + np.exp(-float(np.asarray(M_raw)))) * (MAX_DEGREE - 1) + 1.0
    coef = {(16, 16, 16, 16): float(W[0, 0]) + bval}
    col = 1
    for d in range(1, MAX_DEGREE + 1):
        w_d = 1.0 / (1.0 + np.exp(-10.0 * (M - d + 0.5)))
        for t in combinations_with_replacement(range(D), d):
            tup = tuple(sorted(t + (16,) * (4 - d)))
            coef[tup] = float(W[0, col]) * w_d
            col += 1
    assert col == P_FULL
    S4 = np.zeros((DA * DA, DA * DA), np.float64)
    for tup, c in coef.items():
        perms = set(permutations(tup))
        v = c / len(perms)
        for (a, b2, c2, d2) in perms:
            S4[a * DA + b2, c2 * DA + d2] += v
    lookup = {}
    for p, (a, c) in enumerate(zip(PAIR_A, PAIR_B)):
        lookup[(a, c)] = p
        lookup[(c, a)] = p
    B = np.zeros((DA * DA, PD))
    for j in range(DA):
        for k in range(DA):
            B[j * DA + k, lookup[(j, k)]] = 1.0
    return B.T @ S4 @ B  # float64 [153, 153]


def _build_const(S):
    """Eigendecompose S153 and pack U + sign vectors into one [128, 310] fp16."""
    lam, V = np.linalg.eigh(S)
    U = (V * np.sqrt(np.abs(lam))[None, :]).astype(np.float16)  # columns scaled
    sign = np.sign(lam).astype(np.float16)
    cst = np.zeros((K0, NCOL), np.float16)
    cst[:, :PD] = U[:K0]                    # u0 [128, 153]
    cst[:K1, PD:2 * PD] = U[K0:]            # u1 [25, 153]
    cst[:, 306] = sign[:K0]                 # sg0
    for gp in range(3):                     # sg1 banded at 32*gp
        cst[32 * gp : 32 * gp + K1, 307] = sign[K0:]
    return cst


def _build_nc():
    nc = bacc.Bacc("TRN2", target_bir_lowering=False, debug=False, enable_asserts=False)
    f16 = mybir.dt.float16
    f32 = mybir.dt.float32
    # packed pair operands: [:, 0, :] = X9 rows, [:, 1, :] = X9s rows
    pa_d = nc.dram_tensor("pa", [K0, 2, B_CORE], f16, kind="ExternalInput").ap()
    pb_d = nc.dram_tensor("pb", [K1, 2, B_CORE], f16, kind="ExternalInput").ap()
    cst_d = nc.dram_tensor("cst", [K0, NCOL], f16, kind="ExternalInput").ap()
    out_d = nc.dram_tensor("out", [N_GROUPS, GW], f32, kind="ExternalOutput").ap()

    with tile.TileContext(nc) as tc:
        with (
            tc.tile_pool(name="const", bufs=1) as const_pool,
            tc.tile_pool(name="xx", bufs=6) as xx_pool,
            tc.tile_pool(name="p0", bufs=8) as p0_pool,
            tc.tile_pool(name="z0ps", bufs=4, space="PSUM") as z0_pool,
            tc.tile_pool(name="z1ps", bufs=1, space="PSUM") as z1_pool,
            tc.tile_pool(name="qps", bufs=1, space="PSUM") as q_pool,
        ):
            # resident constants + staged inputs
            cst = const_pool.tile([K0, NCOL], f16)
            u0 = cst[:, 0:PD]
            u1 = cst[:K1, PD:2 * PD]
            u1p = cst[:, PD:2 * PD]   # rows 25..127 are zeros (host-padded)
            sg0 = cst[:, 306:307]
            sg1 = cst[:, 307:308]

            pa = const_pool.tile([K0, 2, B_CORE], f16)
            pb = const_pool.tile([K1, 2, B_CORE], f16)
            # progressive slices sized so slice k lands before DVE consumes it;
            # cst rides after the first pair (needed by the first matmul)
            nc.sync.dma_start(out=cst[:], in_=cst_d[:])
            # window-aligned slices: each window's data in one DMA pair, so no
            # mid-window stalls and only 7 DMA instructions total
            for lo, hi in ((0, 2 * GW), (2 * GW, 5 * GW), (5 * GW, 8 * GW)):
                nc.sync.dma_start(out=pa[:, :, lo:hi], in_=pa_d[:, :, lo:hi])
                nc.sync.dma_start(out=pb[:, :, lo:hi], in_=pb_d[:, :, lo:hi])

            # pre-zero the xx1 rotation buffers' tail rows once: the padded
            # K=128 matmuls read rows 25..127 against zero weights
            for i in range(6):
                xx1z = xx_pool.tile([K0, GW], f16, name="xx1")
                nc.gpsimd.memset(xx1z[:], 0.0)

            # warm the sigmoid table-set early (Square co-resides in every set)
            warm = const_pool.tile([1, 1], f32)
            nc.vector.memset(warm[:], 0.0)
            nc.scalar.activation(warm[:], warm[:], mybir.ActivationFunctionType.Sigmoid)

            # banded PSUM tiles (bands at 0/32/64); zero once so gaps are defined
            z1_tiles = [z1_pool.tile([64 + K1, GW], f32, name=f"z1t{i}") for i in range(2)]
            q_tiles = [q_pool.tile([65, GW], f32, name=f"qt{i}") for i in range(2)]
            for t in z1_tiles + q_tiles:
                nc.vector.memset(t[:], 0.0)

            p1_tiles = [const_pool.tile([64 + K1, GW], f16, name=f"p1t{i}") for i in range(2)]
            o_all = const_pool.tile([65, len(WINDOWS) * GW], f32)

            # warm-up matmuls bridge the input-DMA wait so the PE clock is
            # ramped (and stays ramped) when the first real matmul issues
            warm_ps = z0_pool.tile([K0, 256], f32, name="z0_ps")
            for _ in range(14):
                nc.tensor.matmul(out=warm_ps[:], lhsT=cst[:, :K0],
                                 rhs=cst[:, :256], start=True, stop=True,
                                 skip_group_check=True)
            for _ in range(8):
                nc.tensor.matmul(out=warm_ps[:, :128], lhsT=cst[:, :K0],
                                 rhs=cst[:, :128], start=True, stop=True,
                                 skip_group_check=True)

            p0_win = {}

            def window_mms(wi, prev_wi):
                """Window wi's matmuls in same-tile-size blocks (each tile-size
                switch costs ~95ns of PE pipeline restart), with the previous
                window's q-matmuls folded into the matching blocks."""
                win = WINDOWS[wi]
                z1_ps = z1_tiles[wi % 2]
                z0s, xxs = [], []
                for gp, g in enumerate(win):
                    sl = slice(g * GW, (g + 1) * GW)
                    xx0 = xx_pool.tile([K0, GW], f16, name="xx0")
                    xx1 = xx_pool.tile([K0, GW], f16, name="xx1")
                    nc.vector.tensor_tensor(
                        out=xx0[:], in0=pa[:, 0, sl], in1=pa[:, 1, sl],
                        op=mybir.AluOpType.mult)
                    nc.vector.tensor_tensor(
                        out=xx1[:K1, :], in0=pb[:, 0, sl], in1=pb[:, 1, sl],
                        op=mybir.AluOpType.mult)
                    xxs.append((xx0, xx1))
                    z0s.append(z0_pool.tile([K0, GW], f32, name="z0_ps"))
                if prev_wi is not None:
                    pwin = WINDOWS[prev_wi]
                    pq_ps = q_tiles[prev_wi % 2]
                    pp1 = p1_tiles[prev_wi % 2]
                # block A: prev window's q K1-parts, tile (32,32) -- opens q
                if prev_wi is not None:
                    for gp in range(len(pwin)):
                        band = slice(32 * gp, 32 * gp + K1)
                        nc.tensor.matmul(out=pq_ps[32 * gp : 32 * gp + 1, :],
                                         lhsT=sg1[band, :], rhs=pp1[band, :],
                                         start=True, stop=False,
                                         skip_group_check=True)
                def blk_z0():
                    for gp in range(len(win)):
                        nc.tensor.matmul(out=z0s[gp][:], lhsT=u0[:, :K0],
                                         rhs=xxs[gp][0][:], start=True, stop=False,
                                         skip_group_check=True)
                    for gp in range(len(win)):
                        nc.tensor.matmul(out=z0s[gp][:], lhsT=u1p[:, :K0],
                                         rhs=xxs[gp][1][:], start=False, stop=True,
                                         skip_group_check=True)

                def blk_z1():
                    for gp in range(len(win)):
                        band = slice(32 * gp, 32 * gp + K1)
                        nc.tensor.matmul(out=z1_ps[band, :], lhsT=u1p[:, K0:],
                                         rhs=xxs[gp][1][:], start=True, stop=False,
                                         skip_group_check=True)
                    for gp in range(len(win)):
                        band = slice(32 * gp, 32 * gp + K1)
                        nc.tensor.matmul(out=z1_ps[band, :], lhsT=u0[:, K0:],
                                         rhs=xxs[gp][0][:], start=False, stop=True,
                                         skip_group_check=True)

                def blk_pq0():
                    if prev_wi is not None:
                        for gp in range(len(pwin)):
                            nc.tensor.matmul(out=pq_ps[32 * gp : 32 * gp + 1, :],
                                             lhsT=sg0, rhs=p0_win.pop(pwin[gp])[:],
                                             start=False, stop=True,
                                             skip_group_check=True)

                if wi == len(WINDOWS) - 1:
                    # last window: z1 first so its ScalarE square (gating the
                    # drain-phase q-matmuls) starts as early as possible
                    blk_z1(); blk_z0(); blk_pq0()
                else:
                    blk_z0(); blk_z1(); blk_pq0()
                # ScalarE: square z1 first in late windows (drain-phase q needs
                # it), then this window's z0 chunks
                late = wi == len(WINDOWS) - 1
                if late:
                    nc.scalar.activation(p1_tiles[wi % 2][:], z1_ps[:],
                                         mybir.ActivationFunctionType.Square)
                for gp, g in enumerate(win):
                    p0_sb = p0_pool.tile([K0, GW], f16, name="p0")
                    nc.scalar.activation(p0_sb[:], z0s[gp][:],
                                         mybir.ActivationFunctionType.Square)
                    p0_win[g] = p0_sb
                if not late:
                    nc.scalar.activation(p1_tiles[wi % 2][:], z1_ps[:],
                                         mybir.ActivationFunctionType.Square)
                # prev window's sigmoid after its q bands close, then store
                if prev_wi is not None:
                    nc.scalar.activation(
                        o_all[:, prev_wi * GW:(prev_wi + 1) * GW], pq_ps[:],
                        mybir.ActivationFunctionType.Sigmoid)
                    for gp2 in range(len(pwin)):
                        nc.sync.dma_start(
                            out=out_d[pwin[gp2] : pwin[gp2] + 1, :],
                            in_=o_all[32 * gp2 : 32 * gp2 + 1,
                                      prev_wi * GW:(prev_wi + 1) * GW])

            def finish_last(wi):
                """Final window's q-matmuls + sigmoid (no next window to ride)."""
                win = WINDOWS[wi]
                q_ps = q_tiles[wi % 2]
                p1_sb = p1_tiles[wi % 2]
                for gp in range(len(win)):
                    band = slice(32 * gp, 32 * gp + K1)
                    nc.tensor.matmul(out=q_ps[32 * gp : 32 * gp + 1, :],
                                     lhsT=sg1[band, :], rhs=p1_sb[band, :],
                                     start=True, stop=False, skip_group_check=True)
                for gp in range(len(win)):
                    nc.tensor.matmul(out=q_ps[32 * gp : 32 * gp + 1, :],
                                     lhsT=sg0, rhs=p0_win.pop(win[gp])[:],
                                     start=False, stop=True, skip_group_check=True)
                nc.scalar.activation(o_all[:, wi * GW:(wi + 1) * GW], q_ps[:],
                                     mybir.ActivationFunctionType.Sigmoid)
                for gp2 in range(len(win)):
                    nc.sync.dma_start(
                        out=out_d[win[gp2] : win[gp2] + 1, :],
                        in_=o_all[32 * gp2 : 32 * gp2 + 1,
                                  wi * GW:(wi + 1) * GW])

            for wi in range(len(WINDOWS)):
                window_mms(wi, wi - 1 if wi > 0 else None)
            finish_last(len(WINDOWS) - 1)

    nc.compile()
    return nc


_NC_CACHE = None


def _pack_x(x):
    """Per-core packed fp16 pair operands: pa [128, 2, B] rows 0..127 of
    (X9|X9s), pb [25, 2, B] rows 128..152."""
    x1 = np.concatenate([x, np.ones((x.shape[0], 1), np.float32)], axis=1)
    x1t = np.ascontiguousarray(x1.reshape(N_CORES, B_CORE, DA).transpose(0, 2, 1))
    x1t = x1t.astype(np.float16)  # [C, 17, B_CORE]
    X9 = x1t[:, PAIR_A, :]   # [C, 153, B]
    X9s = x1t[:, PAIR_B, :]
    pa = np.ascontiguousarray(np.stack([X9[:, :K0], X9s[:, :K0]], axis=2))
    pb = np.ascontiguousarray(np.stack([X9[:, K0:], X9s[:, K0:]], axis=2))
    return pa, pb  # [C,128,2,B], [C,25,2,B]


def _make_in_maps(x, W, b, M_raw):
    x = np.asarray(x, np.float32)
    pa, pb = _pack_x(x)
    cst = _build_const(_build_s153(W, b, M_raw))
    return [{"pa": pa[i], "pb": pb[i], "cst": cst} for i in range(N_CORES)]


def kernel(x, W, b, M_raw):
    global _NC_CACHE
    in_maps = _make_in_maps(x, W, b, M_raw)
    if _NC_CACHE is None:
        _NC_CACHE = _build_nc()
    nc = _NC_CACHE
    res = bass_utils.run_bass_kernel_spmd(nc, in_maps, core_ids=list(range(N_CORES)))
    out = np.concatenate([res.results[i]["out"].reshape(B_CORE) for i in range(N_CORES)])
    return out.reshape(BATCH, 1).astype(np.float32)


if __name__ == "__main__":
    x = np.random.randn(BATCH, D).astype(np.float32)
    W = (np.random.randn(1, P_FULL) * 0.02).astype(np.float32)
    b = np.zeros((1,), np.float32)
    M_raw = np.zeros((), np.float32)
    out = kernel(x, W, b, M_raw)
    print("out shape:", out.shape, out.dtype, out[:4, 0])


# revision 35
# speedup vs baseline: 1.0538x; 1.0200x over previous
"""Trainium2 Bass kernel for nn_LogisticRegressionModel (polynomial-feature logistic regression).

Math: reference computes sigmoid(poly_features(x) @ W.T + b), poly features = all
monomials of x (dim 16) up to degree 4, soft-weighted per degree. Every monomial
embeds as a degree-4 monomial over x1 = [x, 1] (17 symbols). Folding W, b, M_raw
into a symmetric quartic matrix over the 153 wrap-encoded unordered pairs
p=(d,j) <-> {j,(j+d)%17}: logit_i = XX_i^T S153 XX_i with XX_i[p] = x1_i[a] x1_i[b].

Device pipeline (feature-major layout, per 512-sample group, fp16 / fp32 PSUM):
  XX^T[p, s] = X9[p, s] * X9s[p, s]        -- DVE, X9/X9s host-replicated x1^T rows
  Z = U^T XX^T  (S153 = U diag(sign) U^T)  -- matmuls, stationary U resident
  P = Z^2                                  -- ScalarE Square, PSUM -> SBUF
  q = sign^T P                             -- matmuls (K=128 + K=25 bands)
  out = sigmoid(q)                         -- ScalarE, batched per 3-group window
Perf notes (tuned against the TRN2 timeline cost model):
  * 153 = 128 + 25. The 25-row tail chunks are zero-PADDED to K=128 (host pads
    U with zero rows; xx1 tile tails zeroed once by GPSIMD) so consecutive
    matmuls keep one PE tile_size -- a tile-size switch costs ~95ns of PE
    pipeline restart and blocks the 1.2->2.4 GHz ramp.
  * Matmuls are emitted in same-tile-size blocks; the 25-row z1/q outputs pack
    into 32-partition PSUM bands at bases 0/32/64 (3 groups per window), and
    band-tiled (32,32)/(128,32) matmuls pack in the PE array (~4ns apart).
  * Each window's q-matmuls/sigmoid are deferred into the NEXT window so
    ScalarE squares never stall the PE; outputs stream per window.
  * 10 warm-up matmuls on the constant tile run during the input DMA wait to
    pre-ramp the PE clock. DMA instruction count is minimized (packed pair
    tensors, 4+1 slices, packed constants) -- each DMA costs ~625ns on the
    serialized HWDGE pipe plus bytes at 360 GB/s on the shared DMA device.

Sharding: pure data-parallel over the batch, 4096 rows per core x 8 cores.
"""
import sys
import numpy as np
from itertools import combinations_with_replacement, permutations

sys.path.insert(0, "/opt/trn_rl_repo")

import concourse.bass as bass
import concourse.bacc as bacc
import concourse.tile as tile
from concourse import mybir
from concourse import bass_utils

BATCH = 32768
D = 16
DA = 17            # features + constant slot
ND = 9             # wrap distances 0..8
PD = ND * DA       # 153 unordered pairs
K0, K1 = 128, PD - 128
MAX_DEGREE = 4
N_CORES = 8
B_CORE = BATCH // N_CORES   # 4096
GW = 512                    # group width (PSUM bank = 512 fp32)
N_GROUPS = B_CORE // GW     # 8
WINDOWS = [[0, 1], [2, 3, 4], [5, 6, 7]]
NCOL = 310                  # packed const columns: 153 u0 | 153 u1 | sg0 | sg1
P_FULL = 1 + sum(
    len(list(combinations_with_replacement(range(D), d))) for d in range(1, MAX_DEGREE + 1)
)

# wrap pair tables (row p of XX^T multiplies x1 rows PAIR_A[p] * PAIR_B[p])
PAIR_A = np.array([j for d in range(ND) for j in range(DA)], np.int64)
PAIR_B = np.array([(j + d) % DA for d in range(ND) for j in range(DA)], np.int64)


def _build_s153(W, b, M_raw):
    """Fold W, b and the soft degree weights into the symmetric quartic
    coefficient matrix over the 153 wrap-encoded unordered pairs."""
    W = np.asarray(W, np.float64)
    bval = float(np.asarray(b).reshape(-1)[0])
    M = 1.0 / (1.0 + np.exp(-float(np.asarray(M_raw)))) * (MAX_DEGREE - 1) + 1.0
    coef = {(16, 16, 16, 16): float(W[0, 0]) + bval}
    col = 1
    for d in range(1, MAX_DEGREE + 1):
        w_d = 1.0 / (1.0 + np.exp(-10.0 * (M - d + 0.5)))
        for t in combinations_with_replacement(range(D), d):
            tup = tuple(sorted(t + (16,) * (4 - d)))
            coef[tup] = float(W[0, col]) * w_d
            col += 1
    assert col == P_FULL
    S4 = np.zeros((DA * DA, DA * DA), np.float64)
    for tup, c in coef.items():
        perms = set(permutations(tup))
        v = c / len(perms)
        for (a, b2, c2, d2) in perms:
            S4[a * DA + b2, c2 * DA + d2] += v
    lookup = {}
    for p, (a, c) in enumerate(zip(PAIR_A, PAIR_B)):
        lookup[(a, c)] = p
        lookup[(c, a)] = p
    B = np.zeros((DA * DA, PD))
    for j in range(DA):
        for k in range(DA):
            B[j * DA + k, lookup[(j, k)]] = 1.0
    return B.T @ S4 @ B  # float64 [153, 153]


def _build_const(S):
    """Eigendecompose S153 and pack U + sign vectors into one [128, 310] fp16."""
    lam, V = np.linalg.eigh(S)
    U = (V * np.sqrt(np.abs(lam))[None, :]).astype(np.float16)  # columns scaled
    sign = np.sign(lam).astype(np.float16)
    cst = np.zeros((K0, NCOL), np.float16)
    cst[:, :PD] = U[:K0]                    # u0 [128, 153]
    cst[:K1, PD:2 * PD] = U[K0:]            # u1 [25, 153]
    cst[:, 306] = sign[:K0]                 # sg0
    for gp in range(3):                     # sg1 banded at 32*gp
        cst[32 * gp : 32 * gp + K1, 307] = sign[K0:]
    return cst


def _build_nc():
    nc = bacc.Bacc("TRN2", target_bir_lowering=False, debug=False, enable_asserts=False)
    f16 = mybir.dt.float16
    f32 = mybir.dt.float32
    # packed pair operands: [:, 0, :] = X9 rows, [:, 1, :] = X9s rows
    pa_d = nc.dram_tensor("pa", [K0, 2, B_CORE], f16, kind="ExternalInput").ap()
    pb_d = nc.dram_tensor("pb", [K1, 2, B_CORE], f16, kind="ExternalInput").ap()
    cst_d = nc.dram_tensor("cst", [K0, NCOL], f16, kind="ExternalInput").ap()
    out_d = nc.dram_tensor("out", [N_GROUPS, GW], f32, kind="ExternalOutput").ap()

    with tile.TileContext(nc) as tc:
        with (
            tc.tile_pool(name="const", bufs=1) as const_pool,
            tc.tile_pool(name="xx", bufs=6) as xx_pool,
            tc.tile_pool(name="p0", bufs=8) as p0_pool,
            tc.tile_pool(name="z0ps", bufs=4, space="PSUM") as z0_pool,
            tc.tile_pool(name="z1ps", bufs=1, space="PSUM") as z1_pool,
            tc.tile_pool(name="qps", bufs=1, space="PSUM") as q_pool,
        ):
            # resident constants + staged inputs
            cst = const_pool.tile([K0, NCOL], f16)
            u0 = cst[:, 0:PD]
            u1 = cst[:K1, PD:2 * PD]
            u1p = cst[:, PD:2 * PD]   # rows 25..127 are zeros (host-padded)
            sg0 = cst[:, 306:307]
            sg1 = cst[:, 307:308]

            pa = const_pool.tile([K0, 2, B_CORE], f16)
            pb = const_pool.tile([K1, 2, B_CORE], f16)
            # progressive slices sized so slice k lands before DVE consumes it;
            # cst rides after the first pair (needed by the first matmul)
            nc.sync.dma_start(out=cst[:], in_=cst_d[:])
            # window-aligned slices: each window's data in one DMA pair, so no
            # mid-window stalls and only 7 DMA instructions total
            for lo, hi in ((0, 2 * GW), (2 * GW, 5 * GW), (5 * GW, 8 * GW)):
                nc.sync.dma_start(out=pa[:, :, lo:hi], in_=pa_d[:, :, lo:hi])
                nc.sync.dma_start(out=pb[:, :, lo:hi], in_=pb_d[:, :, lo:hi])

            # pre-zero the xx1 rotation buffers' tail rows once: the padded
            # K=128 matmuls read rows 25..127 against zero weights
            for i in range(6):
                xx1z = xx_pool.tile([K0, GW], f16, name="xx1")
                nc.gpsimd.memset(xx1z[:], 0.0)

            # warm the sigmoid table-set early (Square co-resides in every set)
            warm = const_pool.tile([1, 1], f32)
            nc.vector.memset(warm[:], 0.0)
            nc.scalar.activation(warm[:], warm[:], mybir.ActivationFunctionType.Sigmoid)

            # banded PSUM tiles (bands at 0/32/64); zero once so gaps are defined
            z1_tiles = [z1_pool.tile([64 + K1, GW], f32, name=f"z1t{i}") for i in range(2)]
            q_tiles = [q_pool.tile([65, GW], f32, name=f"qt{i}") for i in range(2)]
            for t in z1_tiles + q_tiles:
                nc.vector.memset(t[:], 0.0)

            p1_tiles = [const_pool.tile([64 + K1, GW], f16, name=f"p1t{i}") for i in range(2)]
            o_all = const_pool.tile([65, len(WINDOWS) * GW], f32)

            # warm-up matmuls bridge the input-DMA wait so the PE clock is
            # ramped (and stays ramped) when the first real matmul issues
            warm_ps = z0_pool.tile([K0, 256], f32, name="z0_ps")
            for _ in range(11):
                nc.tensor.matmul(out=warm_ps[:], lhsT=cst[:, :K0],
                                 rhs=cst[:, :256], start=True, stop=True,
                                 skip_group_check=True)
            for _ in range(8):
                nc.tensor.matmul(out=warm_ps[:, :128], lhsT=cst[:, :K0],
                                 rhs=cst[:, :128], start=True, stop=True,
                                 skip_group_check=True)

            p0_win = {}

            def window_mms(wi, prev_wi):
                """Window wi's matmuls in same-tile-size blocks (each tile-size
                switch costs ~95ns of PE pipeline restart), with the previous
                window's q-matmuls folded into the matching blocks."""
                win = WINDOWS[wi]
                z1_ps = z1_tiles[wi % 2]
                z0s, xxs = [], []
                for gp, g in enumerate(win):
                    sl = slice(g * GW, (g + 1) * GW)
                    xx0 = xx_pool.tile([K0, GW], f16, name="xx0")
                    xx1 = xx_pool.tile([K0, GW], f16, name="xx1")
                    nc.vector.tensor_tensor(
                        out=xx0[:], in0=pa[:, 0, sl], in1=pa[:, 1, sl],
                        op=mybir.AluOpType.mult)
                    nc.vector.tensor_tensor(
                        out=xx1[:K1, :], in0=pb[:, 0, sl], in1=pb[:, 1, sl],
                        op=mybir.AluOpType.mult)
                    xxs.append((xx0, xx1))
                    z0s.append(z0_pool.tile([K0, GW], f32, name="z0_ps"))
                if prev_wi is not None:
                    pwin = WINDOWS[prev_wi]
                    pq_ps = q_tiles[prev_wi % 2]
                    pp1 = p1_tiles[prev_wi % 2]
                # block A: prev window's q K1-parts, tile (32,32) -- opens q
                if prev_wi is not None:
                    for gp in range(len(pwin)):
                        band = slice(32 * gp, 32 * gp + K1)
                        nc.tensor.matmul(out=pq_ps[32 * gp : 32 * gp + 1, :],
                                         lhsT=sg1[band, :], rhs=pp1[band, :],
                                         start=True, stop=False,
                                         skip_group_check=True)
                def blk_z0():
                    for gp in range(len(win)):
                        nc.tensor.matmul(out=z0s[gp][:], lhsT=u0[:, :K0],
                                         rhs=xxs[gp][0][:], start=True, stop=False,
                                         skip_group_check=True)
                    for gp in range(len(win)):
                        nc.tensor.matmul(out=z0s[gp][:], lhsT=u1p[:, :K0],
                                         rhs=xxs[gp][1][:], start=False, stop=True,
                                         skip_group_check=True)

                def blk_z1():
                    for gp in range(len(win)):
                        band = slice(32 * gp, 32 * gp + K1)
                        nc.tensor.matmul(out=z1_ps[band, :], lhsT=u1p[:, K0:],
                                         rhs=xxs[gp][1][:], start=True, stop=False,
                                         skip_group_check=True)
                    for gp in range(len(win)):
                        band = slice(32 * gp, 32 * gp + K1)
                        nc.tensor.matmul(out=z1_ps[band, :], lhsT=u0[:, K0:],
                                         rhs=xxs[gp][0][:], start=False, stop=True,
                                         skip_group_check=True)

                def blk_pq0():
                    if prev_wi is not None:
                        for gp in range(len(pwin)):
                            nc.tensor.matmul(out=pq_ps[32 * gp : 32 * gp + 1, :],
                                             lhsT=sg0, rhs=p0_win.pop(pwin[gp])[:],
                                             start=False, stop=True,
                                             skip_group_check=True)

                if wi == len(WINDOWS) - 1:
                    # last window: z1 first so its ScalarE square (gating the
                    # drain-phase q-matmuls) starts as early as possible
                    blk_z1(); blk_z0(); blk_pq0()
                else:
                    blk_z0(); blk_z1(); blk_pq0()
                # ScalarE: square z1 first in late windows (drain-phase q needs
                # it), then this window's z0 chunks
                late = wi == len(WINDOWS) - 1
                if late:
                    nc.scalar.activation(p1_tiles[wi % 2][:], z1_ps[:],
                                         mybir.ActivationFunctionType.Square)
                for gp, g in enumerate(win):
                    p0_sb = p0_pool.tile([K0, GW], f16, name="p0")
                    nc.scalar.activation(p0_sb[:], z0s[gp][:],
                                         mybir.ActivationFunctionType.Square)
                    p0_win[g] = p0_sb
                if not late:
                    nc.scalar.activation(p1_tiles[wi % 2][:], z1_ps[:],
                                         mybir.ActivationFunctionType.Square)
                # prev window's sigmoid after its q bands close, then store
                if prev_wi is not None:
                    nc.scalar.activation(
                        o_all[:, prev_wi * GW:(prev_wi + 1) * GW], pq_ps[:],
                        mybir.ActivationFunctionType.Sigmoid)
                    for gp2 in range(len(pwin)):
                        nc.sync.dma_start(
                            out=out_d[pwin[gp2] : pwin[gp2] + 1, :],
                            in_=o_all[32 * gp2 : 32 * gp2 + 1,
                                      prev_wi * GW:(prev_wi + 1) * GW])

            def finish_last(wi):
                """Final window's q-matmuls + sigmoid (no next window to ride)."""
                win = WINDOWS[wi]
                q_ps = q_tiles[wi % 2]
                p1_sb = p1_tiles[wi % 2]
                for gp in range(len(win)):
                    band = slice(32 * gp, 32 * gp + K1)
                    nc.tensor.matmul(out=q_ps[32 * gp : 32 * gp + 1, :],
                                     lhsT=sg1[band, :], rhs=p1_sb[band, :],
                                     start=True, stop=False, skip_group_check=True)
                for gp in range(len(win)):
                    nc.tensor.matmul(out=q_ps[32 * gp : 32 * gp + 1, :],
                                     lhsT=sg0, rhs=p0_win.pop(win[gp])[:],
                                     start=False, stop=True, skip_group_check=True)
                nc.scalar.activation(o_all[:, wi * GW:(wi + 1) * GW], q_ps[:],
                                     mybir.ActivationFunctionType.Sigmoid)
                for gp2 in range(len(win)):
                    nc.sync.dma_start(
                        out=out_d[win[gp2] : win[gp2] + 1, :],
                        in_=o_all[32 * gp2 : 32 * gp2 + 1,
                                  wi * GW:(wi + 1) * GW])

            for wi in range(len(WINDOWS)):
                window_mms(wi, wi - 1 if wi > 0 else None)
            finish_last(len(WINDOWS) - 1)

    nc.compile()
    return nc


_NC_CACHE = None


def _pack_x(x):
    """Per-core packed fp16 pair operands: pa [128, 2, B] rows 0..127 of
    (X9|X9s), pb [25, 2, B] rows 128..152."""
    x1 = np.concatenate([x, np.ones((x.shape[0], 1), np.float32)], axis=1)
    x1t = np.ascontiguousarray(x1.reshape(N_CORES, B_CORE, DA).transpose(0, 2, 1))
    x1t = x1t.astype(np.float16)  # [C, 17, B_CORE]
    X9 = x1t[:, PAIR_A, :]   # [C, 153, B]
    X9s = x1t[:, PAIR_B, :]
    pa = np.ascontiguousarray(np.stack([X9[:, :K0], X9s[:, :K0]], axis=2))
    pb = np.ascontiguousarray(np.stack([X9[:, K0:], X9s[:, K0:]], axis=2))
    return pa, pb  # [C,128,2,B], [C,25,2,B]


def _make_in_maps(x, W, b, M_raw):
    x = np.asarray(x, np.float32)
    pa, pb = _pack_x(x)
    cst = _build_const(_build_s153(W, b, M_raw))
    return [{"pa": pa[i], "pb": pb[i], "cst": cst} for i in range(N_CORES)]


def kernel(x, W, b, M_raw):
    global _NC_CACHE
    in_maps = _make_in_maps(x, W, b, M_raw)
    if _NC_CACHE is None:
        _NC_CACHE = _build_nc()
    nc = _NC_CACHE
    res = bass_utils.run_bass_kernel_spmd(nc, in_maps, core_ids=list(range(N_CORES)))
    out = np.concatenate([res.results[i]["out"].reshape(B_CORE) for i in range(N_CORES)])
    return out.reshape(BATCH, 1).astype(np.float32)


if __name__ == "__main__":
    x = np.random.randn(BATCH, D).astype(np.float32)
    W = (np.random.randn(1, P_FULL) * 0.02).astype(np.float32)
    b = np.zeros((1,), np.float32)
    M_raw = np.zeros((), np.float32)
    out = kernel(x, W, b, M_raw)
    print("out shape:", out.shape, out.dtype, out[:4, 0])
